# revision 29
# baseline (speedup 1.0000x reference)
"""Multi-head attention (B=4, S=2048, d_model=1024, H=16) on 8 TRN2 NeuronCores.

Sharding: core c handles batch c//2 and query rows [1024*(c%2), +1024).
Each core redundantly projects K/V for its batch (no collectives) and
produces a disjoint [1024, 1024] slice of the output.

v3 structure (vs v2 baseline):
  - ALL projection work (V chunks + K/Q of pairs 1..7) is emitted as
    fine-grained micro-units (2 matmuls each) pumped 1-3 per attention
    iteration, so the ACT engine (exp) is never starved by long PE-only
    projection bursts.
  - V projection is interleaved into pair-0's first query-block sweep
    (chunk-pair c ready just before the attnV step that consumes it).
  - kt/qt input DMAs are split across queues so no single 4MB transfer
    gates the first projections.
  - softmax recip broadcast now uses gpsimd partition_broadcast into an
    SBUF tile (replaces the fp16 selector matmul), freeing the kproj
    PSUM slot for the long-held interleaved projection accumulators.
"""

import contextlib

import numpy as np

import bass_rust
import concourse.bass as bass
import concourse.mybir as mybir
import concourse.tile as tile
from concourse.bass_utils import run_bass_kernel_spmd
from concourse.vector_clock import ScopedClock

F32 = mybir.dt.float32
F16 = mybir.dt.float16
BF16 = mybir.dt.bfloat16
AF = mybir.ActivationFunctionType
ADD = mybir.AluOpType.add
MULT = mybir.AluOpType.mult

D_MODEL = 1024
B = 4
S = 2048
N_CORES = 8
QL = 1024  # query rows per core
NPAIR = 8  # head pairs
NK = D_MODEL // 128  # contraction chunks
NT = S // 128  # key chunks
VPW = 65 * 16  # padded V width (64 dims + 1 ones col per head)

# ---------------------------------------------------------------------------
# Workaround for this container's walrus build: each instruction may carry at
# most ONE embedded sync-wait ("Too many sync wait commands" otherwise). Tile
# attaches several; split the extras onto same-engine NOPs placed immediately
# before the instruction (engine queues are in-order => identical semantics).
_MAX_WAITS = 1


def _patched_lower(self, ordered):
    nc = self.nc
    for bb_name, insts in ordered.items():
        new_list = []
        for inst in insts:
            si = inst.sync_info
            waits = list(si.on_wait) if si is not None and si.on_wait else []
            if len(waits) > _MAX_WAITS:
                updates = list(si.on_update) if si.on_update else []
                for w in waits[:-_MAX_WAITS]:
                    nop = bass_rust.InstNoOp(
                        name=nc.get_next_instruction_name(),
                        engine=inst.engine,
                        debug=inst.debug,
                        sync_info=bass_rust.SyncInfo(on_wait=[w], on_update=[]),
                    )
                    new_list.append(nop)
                inst.sync_info = bass_rust.SyncInfo(
                    on_wait=waits[-_MAX_WAITS:], on_update=updates
                )
            new_list.append(inst)
        insts[:] = new_list
    return tile.TileContext._orig_lower_ordered_insts(self, ordered)


def _patched_drain(self, tick_clock, wait_clock):
    probe = self.nc.sync.nop(nofuse=True)
    wait_clock.add_sem_waits(probe.ins, ScopedClock({None: tick_clock.global_clock}))
    si = probe.ins.sync_info
    waits = list(si.on_wait) if si is not None and si.on_wait else []
    if len(waits) > _MAX_WAITS:
        probe.ins.sync_info = bass_rust.SyncInfo(
            on_wait=waits[:_MAX_WAITS], on_update=[]
        )
        for w in waits[_MAX_WAITS:]:
            n = self.nc.sync.nop(nofuse=True)
            n.ins.sync_info = bass_rust.SyncInfo(on_wait=[w], on_update=[])
    self.nc.sync.drain()
    self.nc.all_engine_barrier()
    assert self.sems is not None
    popped = self.nc._tile_sem_poison_stack.pop()
    assert popped is self._sem_poison
    self.nc.clear_and_free_semaphores(list(self.sems.allocated().values()))
    self.nc.all_engine_barrier()


def _install_patch():
    if not hasattr(tile.TileContext, "_orig_lower_ordered_insts"):
        tile.TileContext._orig_lower_ordered_insts = (
            tile.TileContext._lower_ordered_insts
        )
        tile.TileContext._lower_ordered_insts = _patched_lower
        tile.TileContext._drain_and_barrier = _patched_drain


# ---------------------------------------------------------------------------


def _build_bass():
    nc = bass.Bass()
    qt = nc.dram_tensor("qt", [128, NK * QL], BF16, kind="ExternalInput")
    kt = nc.dram_tensor("kt", [128, NK * S], BF16, kind="ExternalInput")
    vt = nc.dram_tensor("vt", [NT, 128, 1024], BF16, kind="ExternalInput")
    wq = nc.dram_tensor("wq", [NPAIR, 128, D_MODEL], BF16, kind="ExternalInput")
    wk = nc.dram_tensor("wk", [NPAIR, 128, D_MODEL], BF16, kind="ExternalInput")
    wv = nc.dram_tensor("wv", [128, NK * VPW], BF16, kind="ExternalInput")
    wo = nc.dram_tensor("wo", [128, NPAIR * D_MODEL], BF16, kind="ExternalInput")
    bqt = nc.dram_tensor("bqt", [128, NK], F32, kind="ExternalInput")
    bkt = nc.dram_tensor("bkt", [128, NK], F32, kind="ExternalInput")
    bvr = nc.dram_tensor("bvr", [128, VPW], BF16, kind="ExternalInput")
    bor = nc.dram_tensor("bor", [128, D_MODEL], BF16, kind="ExternalInput")
    selq = nc.dram_tensor("selq", [2, 4, 128], F16, kind="ExternalInput")
    out = nc.dram_tensor("out", [QL, D_MODEL], F32, kind="ExternalOutput")

    with tile.TileContext(nc) as tc:
        _emit(nc, tc, locals())
    return nc


def _emit(nc, tc, t):
    qt, kt, vt = t["qt"], t["kt"], t["vt"]
    wq, wk, wv, wo = t["wq"], t["wk"], t["wv"], t["wo"]
    bqt, bkt, bvr, bor = t["bqt"], t["bkt"], t["bvr"], t["bor"]
    selq, out = t["selq"], t["out"]

    P = tc.tile_pool

    with (
        P(name="consts", bufs=1) as consts,
        P(name="stg", bufs=3) as stg,
        P(name="sel", bufs=1) as selp,
        P(name="den", bufs=2) as denp,
    ):
        bqt_t = consts.tile([128, NK], F32, tag="bqt")
        bkt_t = consts.tile([128, NK], F32, tag="bkt")
        bvr_t = consts.tile([128, VPW], BF16, tag="bvr")

        # selector tiles for the recip broadcast: sel[qb][r, p] = 1 iff
        # r == 2*(p//64) + qb  (fp16, host constant)
        sel = []
        for qb in range(2):
            st = selp.tile([4, 128], F16, name=f"sel{qb}", tag=f"sel{qb}")
            nc.gpsimd.dma_start(st[:], selq[qb])
            sel.append(st)
        # per-pair softmax-denominator tiles [4, 512] (row r = 2*h + qb)
        den = [None] * NPAIR
        denr = [None] * NPAIR

        with P(name="pv", bufs=1) as pv, P(name="pkq", bufs=1) as pkq:
            # ---- V_pad tiles [c][p, 16*65] bf16; ones col 65h+64 ----------
            v_tiles = [
                pv.tile([128, VPW], BF16, name=f"v{c}", tag=f"v{c}")
                for c in range(NT)
            ]

            xn_tiles = [None] * NPAIR
            xg_tiles = [None] * NPAIR
            KT = [
                pkq.tile([128, S], BF16, name=f"ktg{g}", tag=f"ktg{g}")
                for g in range(NPAIR)
            ]
            QT = [
                pkq.tile([128, QL], BF16, name=f"qtg{g}", tag=f"qtg{g}")
                for g in range(NPAIR)
            ]

            with (
                P(name="expp", bufs=8) as expp,
                P(name="psS", bufs=2, space="PSUM") as psS,
                P(name="psacc", bufs=1, space="PSUM") as psacc,
                P(name="psP", bufs=1, space="PSUM") as psP,
                P(name="sgx", bufs=2) as sgxp,
                P(name="pxg", bufs=2) as pxg,
            ):
                # kq pools opened inside the attention pool scope so they can
                # be closed (SBUF reused) once the last projection is emitted
                kq_es = contextlib.ExitStack()
                kstr = kq_es.enter_context(P(name="kstr", bufs=1))
                qstr = kq_es.enter_context(P(name="qstr", bufs=1))
                wks = kq_es.enter_context(P(name="wks", bufs=1))
                wqs = kq_es.enter_context(P(name="wqs", bufs=1))
                # V-proj staging closes first (exhausted by iteration ~8)
                v_es = contextlib.ExitStack()
                wvp = v_es.enter_context(P(name="wvp", bufs=1))
                vstr = v_es.enter_context(P(name="vstr", bufs=2))

                kt_sb = kstr.tile([128, NK * S], BF16, tag="ktsb")
                qt_sb = qstr.tile([128, NK * QL], BF16, tag="qtsb")
                wv_sb = wvp.tile([128, NK * VPW], BF16, tag="wvsb")
                # spread the big input DMAs across the three DMA-capable
                # queues, ordered by first-use time (kt is consumed by pair-0
                # K proj almost immediately)
                wkg0 = wks.tile([128, D_MODEL], BF16, tag="wks")
                nc.sync.dma_start(wkg0[:], wk[0])
                nc.scalar.dma_start(bkt_t[:], bkt[:])
                nc.scalar.dma_start(bqt_t[:], bqt[:])
                wqg0 = wqs.tile([128, D_MODEL], BF16, tag="wqs")
                # kt by k-chunk, earliest-consumed first, HWDGE queues only
                kq_ = (nc.sync, nc.scalar)
                for k in range(NK):
                    kq_[k % 2].dma_start(
                        kt_sb[:, 2048 * k : 2048 * k + 2048],
                        kt[:, 2048 * k : 2048 * k + 2048],
                    )
                nc.scalar.dma_start(wqg0[:], wq[0])
                nc.sync.dma_start(qt_sb[:, 0:4096], qt[:, 0:4096])
                nc.scalar.dma_start(qt_sb[:, 4096:8192], qt[:, 4096:8192])
                nc.gpsimd.dma_start(bvr_t[:], bvr[:])
                nc.sync.dma_start(wv_sb[:, 0:4160], wv[:, 0:4160])
                nc.scalar.dma_start(wv_sb[:, 4160:8320], wv[:, 4160:8320])

                def emit_kproj(half, g, wkg):
                    # KT[g][:, 1024*half:+1024] = (wk[g].T @ K^T)(half) + bias
                    ps = psS.tile([128, QL], F32, name="ps0", tag="scores")
                    for k in range(NK):
                        for j in range(2):
                            nc.tensor.matmul(
                                ps[:, 512 * j : 512 * j + 512],
                                wkg[:, 128 * k : 128 * k + 128],
                                kt_sb[
                                    :,
                                    2048 * k
                                    + 1024 * half
                                    + 512 * j : 2048 * k
                                    + 1024 * half
                                    + 512 * j
                                    + 512,
                                ],
                                start=(k == 0),
                                stop=(k == NK - 1 and j == 1),
                                skip_group_check=True,
                            )
                    nc.vector.tensor_scalar_add(
                        KT[g][:, 1024 * half : 1024 * half + 1024],
                        ps[:],
                        bkt_t[:, g : g + 1],
                    )

                def emit_qproj(g, wqg):
                    ps = psS.tile([128, QL], F32, name="ps0", tag="scores")
                    for k in range(NK):
                        for j in range(2):
                            nc.tensor.matmul(
                                ps[:, 512 * j : 512 * j + 512],
                                wqg[:, 128 * k : 128 * k + 128],
                                qt_sb[
                                    :, 1024 * k + 512 * j : 1024 * k + 512 * j + 512
                                ],
                                start=(k == 0),
                                stop=(k == NK - 1 and j == 1),
                                skip_group_check=True,
                            )
                    nc.vector.tensor_scalar_add(
                        QT[g][:], ps[:], bqt_t[:, g : g + 1]
                    )

                # pair-0 projections upfront (attention can start ~12us in)
                emit_kproj(0, 0, wkg0)
                emit_kproj(1, 0, wkg0)
                emit_qproj(0, wqg0)

                # ---- fine-grained projection micro-units ------------------
                def v_unit_gen():
                    # one yield = 2 matmuls; a chunk (8 yields) ends with its
                    # bias-add so consumers see a complete v_tiles[c]
                    vts_tiles = {}
                    vts_tiles[0] = vstr.tile([128, 1024], BF16, name="vts0", tag="vts")
                    nc.sync.dma_start(vts_tiles[0][:], vt[0])
                    for c in range(NT):
                        if c + 1 < NT:
                            vts_tiles[c + 1] = vstr.tile(
                                [128, 1024], BF16, name=f"vts{c+1}", tag="vts"
                            )
                            nc.sync.dma_start(vts_tiles[c + 1][:], vt[c + 1])
                        vts = vts_tiles.pop(c)
                        for q in range(4):
                            ps = psP.tile(
                                [128, 260],
                                F32,
                                name="psv",
                                tag="kproj" if q % 2 == 0 else "rep",
                            )
                            for k in range(NK):
                                nc.tensor.matmul(
                                    ps[:],
                                    vts[:, 128 * k : 128 * k + 128],
                                    wv_sb[
                                        :,
                                        VPW * k
                                        + 260 * q : VPW * k
                                        + 260 * q
                                        + 260,
                                    ],
                                    start=(k == 0),
                                    stop=(k == NK - 1),
                                    skip_group_check=True,
                                )
                                if k % 2 == 1 and k < NK - 1:
                                    yield
                            nc.vector.tensor_tensor(
                                v_tiles[c][:, 260 * q : 260 * q + 260],
                                ps[:],
                                bvr_t[:, 260 * q : 260 * q + 260],
                                ADD,
                            )
                            yield
                    v_es.close()

                def pair_unit_gen(g):
                    wkg = wks.tile([128, D_MODEL], BF16, tag="wks")
                    nc.sync.dma_start(wkg[:], wk[g])
                    yield
                    for half in range(2):
                        for j in range(2):
                            ps = psP.tile([128, 512], F32, name="psk", tag="kproj")
                            for k in range(NK):
                                nc.tensor.matmul(
                                    ps[:],
                                    wkg[:, 128 * k : 128 * k + 128],
                                    kt_sb[
                                        :,
                                        2048 * k
                                        + 1024 * half
                                        + 512 * j : 2048 * k
                                        + 1024 * half
                                        + 512 * j
                                        + 512,
                                    ],
                                    start=(k == 0),
                                    stop=(k == NK - 1),
                                    skip_group_check=True,
                                )
                                if k % 2 == 1 and k < NK - 1:
                                    yield
                            nc.vector.tensor_scalar_add(
                                KT[g][
                                    :,
                                    1024 * half + 512 * j : 1024 * half
                                    + 512 * j
                                    + 512,
                                ],
                                ps[:],
                                bkt_t[:, g : g + 1],
                            )
                            yield
                    wqg = wqs.tile([128, D_MODEL], BF16, tag="wqs")
                    nc.scalar.dma_start(wqg[:], wq[g])
                    yield
                    for j in range(2):
                        ps = psP.tile([128, 512], F32, name="psq", tag="kproj")
                        for k in range(NK):
                            nc.tensor.matmul(
                                ps[:],
                                wqg[:, 128 * k : 128 * k + 128],
                                qt_sb[
                                    :, 1024 * k + 512 * j : 1024 * k + 512 * j + 512
                                ],
                                start=(k == 0),
                                stop=(k == NK - 1),
                                skip_group_check=True,
                            )
                            if k % 2 == 1 and k < NK - 1:
                                yield
                        nc.vector.tensor_scalar_add(
                            QT[g][:, 512 * j : 512 * j + 512],
                            ps[:],
                            bqt_t[:, g : g + 1],
                        )
                        yield

                v_gen = v_unit_gen()
                pair_gens = {g: pair_unit_gen(g) for g in range(1, NPAIR)}

                def filler(it):
                    # iteration -> (generator, pumps)
                    if it < 8:
                        # two complete V chunks per iteration (16 yields each)
                        return (v_gen, 32)
                    if it < 16:
                        return (pair_gens[1], 4)
                    g = it // 16 + 1
                    if g < NPAIR:
                        return (pair_gens[g], 2)
                    return (None, 0)

                _STOP = object()

                def pump(gen, n):
                    if gen is None:
                        return
                    for _ in range(n):
                        if next(gen, _STOP) is _STOP:
                            break

                def drain(gen):
                    for _ in gen:
                        pass

                def emit_scores(g, qb, cg):
                    ktg, qtg = KT[g], QT[g]
                    q0 = 512 * qb
                    tiles = [
                        psS.tile([128, QL], F32, name=f"sc{h}", tag="scores")
                        for h in range(2)
                    ]
                    # h0/h1 use disjoint row groups -> can run concurrently;
                    # h-major order gives h1's LDW a 2-MM pull-ahead window
                    for h in range(2):
                        p0 = 64 * h
                        for ci in range(2):
                            c = 2 * cg + ci
                            nc.tensor.matmul(
                                tiles[h][:, 512 * ci : 512 * ci + 512],
                                ktg[p0 : p0 + 64, 128 * c : 128 * c + 128],
                                qtg[p0 : p0 + 64, q0 : q0 + 512],
                                start=True,
                                stop=True,
                                skip_group_check=True,
                            )
                    return tiles

                def emit_spill(g, qb, acc):
                    # X rows -> xg (bf16, SBUF); sum row -> den[g]
                    for h in range(2):
                        if h == 0:
                            # partitions line up: copy straight into xg
                            nc.vector.tensor_copy(
                                xg_tiles[g][0:64, 512 * qb : 512 * qb + 512],
                                acc[h][0:64, :],
                            )
                        else:
                            sx = sgxp.tile([64, 512], BF16, tag="sgx")
                            nc.vector.tensor_copy(sx[:], acc[h][0:64, :])
                            nc.sync.dma_start(
                                xg_tiles[g][64:128, 512 * qb : 512 * qb + 512],
                                sx[:],
                            )
                        sd = sgxp.tile([65, 512], BF16, tag="sgd")
                        nc.vector.tensor_copy(sd[64:65, :], acc[h][64:65, :])
                        nc.sync.dma_start(
                            den[g][2 * h + qb : 2 * h + qb + 1, :], sd[64:65, :]
                        )

                def emit_chain(g):
                    # denr = 1/den ; rep = sel.T @ denr (fp16 broadcast MM)
                    with nc.allow_low_precision(reason="fp16 softmax recips"):
                        nc.vector.reciprocal(denr[g][:], den[g][:])
                    xn = pkq.tile([128, QL], BF16, name=f"xn{g}", tag=f"xn{g}")
                    for qb in range(2):
                        rep = psP.tile([128, 512], F32, name="rep", tag="rep")
                        nc.tensor.matmul(
                            rep[:],
                            sel[qb][:],
                            denr[g][:],
                            start=True,
                            stop=True,
                            skip_group_check=True,
                        )
                        nc.vector.tensor_tensor(
                            xn[:, 512 * qb : 512 * qb + 512],
                            xg_tiles[g][:, 512 * qb : 512 * qb + 512],
                            rep[:],
                            MULT,
                        )
                    xn_tiles[g] = xn

                import os
                if os.environ.get("NOFILL") == "1" or os.environ.get("NOV") == "1":
                    drain(v_gen)
                if os.environ.get("NOFILL") == "1" or os.environ.get("NOPAIR") == "1":
                    for _g in range(1, NPAIR):
                        drain(pair_gens[_g])
                pending_spill = None
                pending_chain = None
                it = 0
                for g in range(NPAIR):
                    if g >= 1:
                        drain(pair_gens[g])  # safety: pair ready before use
                    xg_tiles[g] = pxg.tile(
                        [128, QL], BF16, name=f"xg{g}", tag="xg"
                    )
                    den[g] = denp.tile([4, 512], BF16, name=f"den{g}", tag="den")
                    denr[g] = denp.tile([4, 512], F16, name=f"denr{g}", tag="denr")
                    for qb in range(2):
                        acc = [
                            psacc.tile([65, 512], F32, name="acca", tag="acca"),
                            psacc.tile([65, 512], F32, name="accb", tag="accb"),
                        ]
                        sc_cur = emit_scores(g, qb, 0)
                        if pending_spill is not None:
                            emit_spill(*pending_spill)
                            pending_spill = None
                            if g >= 1 and qb == 1:
                                pending_chain = g - 1
                        for cg in range(NT // 2):
                            gen, n = filler(it)
                            # V chunks must fully precede this iteration's
                            # attnV reads; pair proj units can split around it
                            n1 = n if gen is v_gen else (n + 1) // 2
                            pump(gen, n1)
                            sc_next = (
                                emit_scores(g, qb, cg + 1)
                                if cg + 1 < NT // 2
                                else None
                            )
                            exs = []
                            for h in range(2):
                                ex = expp.tile([128, QL], BF16, tag="exp")
                                nc.scalar.activation(
                                    ex[:], sc_cur[h][:], AF.Exp, scale=0.125
                                )
                                exs.append(ex)
                            for h in range(2):
                                hh = 2 * g + h
                                for ci in range(2):
                                    c = 2 * cg + ci
                                    nc.tensor.matmul(
                                        acc[h][:],
                                        v_tiles[c][:, 65 * hh : 65 * hh + 65],
                                        exs[h][:, 512 * ci : 512 * ci + 512],
                                        start=(c == 0),
                                        stop=(c == NT - 1),
                                        skip_group_check=True,
                                    )

                            if pending_chain is not None and cg == 2:
                                emit_chain(pending_chain)
                                pending_chain = None
                            it += 1
                            sc_cur = sc_next
                        pending_spill = (g, qb, acc)
                emit_spill(*pending_spill)
                for g in range(1, NPAIR):
                    drain(pair_gens[g])
                drain(v_gen)
                kq_es.close()
                emit_chain(7)

            # ---- output projection ---------------------------------------
            with (
                P(name="pwo", bufs=1) as pwo,
                P(name="ps3o", bufs=6, space="PSUM") as ps3o,
            ):
                bor_t = consts.tile([128, D_MODEL], BF16, tag="bor")
                nc.scalar.dma_start(bor_t[:], bor[:])
                wo_sb = pwo.tile([128, NPAIR * D_MODEL], BF16, tag="wosb")
                for g in range(NPAIR):
                    (nc.sync, nc.scalar)[g % 2].dma_start(
                        wo_sb[:, 1024 * g : 1024 * g + 1024],
                        wo[:, 1024 * g : 1024 * g + 1024],
                    )

                qrr = 0
                for m in range(QL // 128):
                    for j in range(2):
                        ps = ps3o.tile([128, 512], F32, tag="oproj")
                        for g in range(NPAIR):
                            nc.tensor.matmul(
                                ps[:],
                                xn_tiles[g][:, 128 * m : 128 * m + 128],
                                wo_sb[
                                    :,
                                    1024 * g + 512 * j : 1024 * g + 512 * j + 512,
                                ],
                                start=(g == 0),
                                stop=(g == NPAIR - 1),
                                skip_group_check=True,
                            )
                        ot = stg.tile([128, 512], F32, tag="outs")
                        nc.vector.tensor_tensor(
                            ot[:], ps[:], bor_t[:, 512 * j : 512 * j + 512], ADD
                        )
                        eng = (nc.sync, nc.scalar)[qrr % 2]
                        qrr += 1
                        eng.dma_start(
                            out[128 * m : 128 * m + 128, 512 * j : 512 * j + 512],
                            ot[:],
                        )


_NC_CACHE = None
LAST_RESULT = None


def _get_nc():
    global _NC_CACHE
    if _NC_CACHE is None:
        _install_patch()
        _NC_CACHE = _build_bass()
    return _NC_CACHE


def kernel(q, k, v, w_q, b_q, w_k, b_k, w_v, b_v, w_o, b_o):
    global LAST_RESULT
    import ml_dtypes

    q = np.asarray(q, np.float32)
    k = np.asarray(k, np.float32)
    v = np.asarray(v, np.float32)

    def _pair_w(w):
        # [in, out] -> [g, 128, 1024]: [g][p, 128k+j] = w[128k+p, 128g+j]
        return np.ascontiguousarray(
            np.asarray(w, np.float32)
            .reshape(NK, 128, NPAIR, 128)
            .transpose(2, 1, 0, 3)
            .reshape(NPAIR, 128, D_MODEL)
        ).astype(ml_dtypes.bfloat16)

    def _chunk_w(w):
        # [in, out] -> [128, 8*1024]: [p, 1024k+o] = w[128k+p, o]
        return np.ascontiguousarray(
            np.asarray(w, np.float32)
            .reshape(NK, 128, D_MODEL)
            .transpose(1, 0, 2)
            .reshape(128, NK * D_MODEL)
        ).astype(ml_dtypes.bfloat16)

    w_q = _pair_w(w_q)
    w_k = _pair_w(w_k)
    # wv padded per chunk to 16 heads x 65 cols; 65th col w=0 (ones come
    # from the padded bias), so the on-chip proj writes are contiguous
    wv_c = np.asarray(w_v, np.float32).reshape(NK, 128, 16, 64)
    wv_pad = np.zeros((128, NK, 16, 65), np.float32)
    wv_pad[:, :, :, 0:64] = wv_c.transpose(1, 0, 2, 3)
    w_v = np.ascontiguousarray(wv_pad.reshape(128, NK * VPW)).astype(
        ml_dtypes.bfloat16
    )
    # wo: [p, 1024g+o] = w_o[128g+p, o] -- same transform (g indexes chunks)
    w_o = _chunk_w(w_o)
    b_q = np.asarray(b_q, np.float32)
    b_k = np.asarray(b_k, np.float32)
    b_v = np.asarray(b_v, np.float32)
    b_o = np.asarray(b_o, np.float32)

    bqt = np.ascontiguousarray(b_q.reshape(NK, 128).T)
    bkt = np.ascontiguousarray(b_k.reshape(NK, 128).T)
    bvr_pad = np.ones((16, 65), np.float32)
    bvr_pad[:, 0:64] = b_v.reshape(16, 64)
    bvr = np.ascontiguousarray(
        np.broadcast_to(bvr_pad.reshape(1, VPW), (128, VPW))
    ).astype(ml_dtypes.bfloat16)
    bor = np.ascontiguousarray(
        np.broadcast_to(b_o[None, :], (128, D_MODEL))
    ).astype(ml_dtypes.bfloat16)
    selq = np.zeros((2, 4, 128), np.float16)
    for qb in range(2):
        selq[qb, qb, 0:64] = 1.0
        selq[qb, 2 + qb, 64:128] = 1.0

    in_maps = []
    for c in range(N_CORES):
        b = c // 2
        r0 = QL * (c % 2)
        # qt: [p, 1024k+t] = q_proj_input^T chunked
        qtc = np.ascontiguousarray(
            q[b, r0 : r0 + QL, :].T.reshape(NK, 128, QL).transpose(1, 0, 2).reshape(
                128, NK * QL
            )
        ).astype(ml_dtypes.bfloat16)
        ktc = np.ascontiguousarray(
            k[b].T.reshape(NK, 128, S).transpose(1, 0, 2).reshape(128, NK * S)
        ).astype(ml_dtypes.bfloat16)
        in_maps.append(
            {
                "qt": qtc,
                "kt": ktc,
                "vt": np.ascontiguousarray(
                    v[b]
                    .T.reshape(8, 128, 16, 128)
                    .transpose(2, 1, 0, 3)
                    .reshape(16, 128, 1024)
                ).astype(ml_dtypes.bfloat16),
                "wq": w_q,
                "wk": w_k,
                "wv": w_v,
                "wo": w_o,
                "bqt": bqt,
                "bkt": bkt,
                "bvr": bvr,
                "bor": bor,
                "selq": selq,
            }
        )

    nc = _get_nc()
    res = run_bass_kernel_spmd(nc, in_maps, list(range(N_CORES)))
    LAST_RESULT = res

    outp = np.empty((B, S, D_MODEL), np.float32)
    for c in range(N_CORES):
        b = c // 2
        r0 = QL * (c % 2)
        outp[b, r0 : r0 + QL, :] = res.results[c]["out"]
    return outp


# revision 30
# speedup vs baseline: 1.3305x; 1.3305x over previous
"""Multi-head attention (B=4, S=2048, d_model=1024, H=16) on 8 TRN2 NeuronCores.

Sharding: core c handles batch c//2 and query rows [1024*(c%2), +1024).
Each core redundantly projects K/V for its batch (no collectives) and
produces a disjoint [1024, 1024] slice of the output.

v3 structure (vs v2 baseline):
  - ALL projection work (V chunks + K/Q of pairs 1..7) is emitted as
    fine-grained micro-units (2 matmuls each) pumped 1-3 per attention
    iteration, so the ACT engine (exp) is never starved by long PE-only
    projection bursts.
  - V projection is interleaved into pair-0's first query-block sweep
    (chunk-pair c ready just before the attnV step that consumes it).
  - kt/qt input DMAs are split across queues so no single 4MB transfer
    gates the first projections.
  - softmax recip broadcast now uses gpsimd partition_broadcast into an
    SBUF tile (replaces the fp16 selector matmul), freeing the kproj
    PSUM slot for the long-held interleaved projection accumulators.
"""

import contextlib

import numpy as np

import bass_rust
import concourse.bass as bass
import concourse.mybir as mybir
import concourse.tile as tile
from concourse.bass_utils import run_bass_kernel_spmd
from concourse.vector_clock import ScopedClock

F32 = mybir.dt.float32
F16 = mybir.dt.float16
BF16 = mybir.dt.bfloat16
AF = mybir.ActivationFunctionType
ADD = mybir.AluOpType.add
MULT = mybir.AluOpType.mult

D_MODEL = 1024
B = 4
S = 2048
N_CORES = 8
QL = 1024  # query rows per core
NPAIR = 8  # head pairs
NK = D_MODEL // 128  # contraction chunks
NT = S // 128  # key chunks
VPW = 65 * 16  # padded V width (64 dims + 1 ones col per head)

# ---------------------------------------------------------------------------
# Workaround for this container's walrus build: each instruction may carry at
# most ONE embedded sync-wait ("Too many sync wait commands" otherwise). Tile
# attaches several; split the extras onto same-engine NOPs placed immediately
# before the instruction (engine queues are in-order => identical semantics).
_MAX_WAITS = 1


def _patched_lower(self, ordered):
    nc = self.nc
    for bb_name, insts in ordered.items():
        new_list = []
        for inst in insts:
            si = inst.sync_info
            waits = list(si.on_wait) if si is not None and si.on_wait else []
            if len(waits) > _MAX_WAITS:
                updates = list(si.on_update) if si.on_update else []
                for w in waits[:-_MAX_WAITS]:
                    nop = bass_rust.InstNoOp(
                        name=nc.get_next_instruction_name(),
                        engine=inst.engine,
                        debug=inst.debug,
                        sync_info=bass_rust.SyncInfo(on_wait=[w], on_update=[]),
                    )
                    new_list.append(nop)
                inst.sync_info = bass_rust.SyncInfo(
                    on_wait=waits[-_MAX_WAITS:], on_update=updates
                )
            new_list.append(inst)
        insts[:] = new_list
    return tile.TileContext._orig_lower_ordered_insts(self, ordered)


def _patched_drain(self, tick_clock, wait_clock):
    probe = self.nc.sync.nop(nofuse=True)
    wait_clock.add_sem_waits(probe.ins, ScopedClock({None: tick_clock.global_clock}))
    si = probe.ins.sync_info
    waits = list(si.on_wait) if si is not None and si.on_wait else []
    if len(waits) > _MAX_WAITS:
        probe.ins.sync_info = bass_rust.SyncInfo(
            on_wait=waits[:_MAX_WAITS], on_update=[]
        )
        for w in waits[_MAX_WAITS:]:
            n = self.nc.sync.nop(nofuse=True)
            n.ins.sync_info = bass_rust.SyncInfo(on_wait=[w], on_update=[])
    self.nc.sync.drain()
    self.nc.all_engine_barrier()
    assert self.sems is not None
    popped = self.nc._tile_sem_poison_stack.pop()
    assert popped is self._sem_poison
    self.nc.clear_and_free_semaphores(list(self.sems.allocated().values()))
    self.nc.all_engine_barrier()


def _install_patch():
    if not hasattr(tile.TileContext, "_orig_lower_ordered_insts"):
        tile.TileContext._orig_lower_ordered_insts = (
            tile.TileContext._lower_ordered_insts
        )
        tile.TileContext._lower_ordered_insts = _patched_lower
        tile.TileContext._drain_and_barrier = _patched_drain


# ---------------------------------------------------------------------------


def _build_bass():
    nc = bass.Bass()
    qt = nc.dram_tensor("qt", [128, NK * QL], BF16, kind="ExternalInput")
    kt = nc.dram_tensor("kt", [128, NK * S], BF16, kind="ExternalInput")
    vt = nc.dram_tensor("vt", [NT, 128, 1024], BF16, kind="ExternalInput")
    wq = nc.dram_tensor("wq", [NPAIR, 128, D_MODEL], BF16, kind="ExternalInput")
    wk = nc.dram_tensor("wk", [NPAIR, 128, D_MODEL], BF16, kind="ExternalInput")
    wv = nc.dram_tensor("wv", [128, NK * VPW], BF16, kind="ExternalInput")
    wo = nc.dram_tensor("wo", [128, NPAIR * D_MODEL], BF16, kind="ExternalInput")
    bqt = nc.dram_tensor("bqt", [128, NK], F32, kind="ExternalInput")
    bkt = nc.dram_tensor("bkt", [128, NK], F32, kind="ExternalInput")
    bvr = nc.dram_tensor("bvr", [128, VPW], BF16, kind="ExternalInput")
    bor = nc.dram_tensor("bor", [128, D_MODEL], BF16, kind="ExternalInput")
    selq = nc.dram_tensor("selq", [2, 4, 128], F16, kind="ExternalInput")
    out = nc.dram_tensor("out", [QL, D_MODEL], F32, kind="ExternalOutput")

    with tile.TileContext(nc) as tc:
        _emit(nc, tc, locals())
    return nc


def _emit(nc, tc, t):
    qt, kt, vt = t["qt"], t["kt"], t["vt"]
    wq, wk, wv, wo = t["wq"], t["wk"], t["wv"], t["wo"]
    bqt, bkt, bvr, bor = t["bqt"], t["bkt"], t["bvr"], t["bor"]
    selq, out = t["selq"], t["out"]

    P = tc.tile_pool

    with (
        P(name="consts", bufs=1) as consts,
        P(name="stg", bufs=3) as stg,
        P(name="sel", bufs=1) as selp,
        P(name="den", bufs=2) as denp,
    ):
        bqt_t = consts.tile([128, NK], F32, tag="bqt")
        bkt_t = consts.tile([128, NK], F32, tag="bkt")
        bvr_t = consts.tile([128, VPW], BF16, tag="bvr")

        # selector tiles for the recip broadcast: sel[qb][r, p] = 1 iff
        # r == 2*(p//64) + qb  (fp16, host constant)
        sel = []
        for qb in range(2):
            st = selp.tile([4, 128], F16, name=f"sel{qb}", tag=f"sel{qb}")
            nc.gpsimd.dma_start(st[:], selq[qb])
            sel.append(st)
        # per-pair softmax-denominator tiles [4, 512] (row r = 2*h + qb)
        den = [None] * NPAIR
        denr = [None] * NPAIR

        with P(name="pv", bufs=1) as pv, P(name="pkq", bufs=1) as pkq:
            # ---- V_pad tiles [c][p, 16*65] bf16; ones col 65h+64 ----------
            v_tiles = [
                pv.tile([128, VPW], BF16, name=f"v{c}", tag=f"v{c}")
                for c in range(NT)
            ]

            xn_tiles = [None] * NPAIR
            xg_tiles = [None] * NPAIR
            KT = [
                pkq.tile([128, S], BF16, name=f"ktg{g}", tag=f"ktg{g}")
                for g in range(NPAIR)
            ]
            QT = [
                pkq.tile([128, QL], BF16, name=f"qtg{g}", tag=f"qtg{g}")
                for g in range(NPAIR)
            ]

            with (
                P(name="expp", bufs=8) as expp,
                P(name="psS", bufs=2, space="PSUM") as psS,
                P(name="psacc", bufs=1, space="PSUM") as psacc,
                P(name="psP", bufs=1, space="PSUM") as psP,
                P(name="sgx", bufs=2) as sgxp,
                P(name="pxg", bufs=2) as pxg,
            ):
                # kq pools opened inside the attention pool scope so they can
                # be closed (SBUF reused) once the last projection is emitted
                kq_es = contextlib.ExitStack()
                kstr = kq_es.enter_context(P(name="kstr", bufs=1))
                qstr = kq_es.enter_context(P(name="qstr", bufs=1))
                wks = kq_es.enter_context(P(name="wks", bufs=1))
                wqs = kq_es.enter_context(P(name="wqs", bufs=1))
                # V-proj staging closes first (exhausted by iteration ~8)
                v_es = contextlib.ExitStack()
                wvp = v_es.enter_context(P(name="wvp", bufs=1))
                vstr = v_es.enter_context(P(name="vstr", bufs=2))

                kt_sb = kstr.tile([128, NK * S], BF16, tag="ktsb")
                qt_sb = qstr.tile([128, NK * QL], BF16, tag="qtsb")
                wv_sb = wvp.tile([128, NK * VPW], BF16, tag="wvsb")
                # spread the big input DMAs across the three DMA-capable
                # queues, ordered by first-use time (kt is consumed by pair-0
                # K proj almost immediately)
                wkg0 = wks.tile([128, D_MODEL], BF16, tag="wks")
                nc.sync.dma_start(wkg0[:], wk[0])
                nc.scalar.dma_start(bkt_t[:], bkt[:])
                nc.scalar.dma_start(bqt_t[:], bqt[:])
                wqg0 = wqs.tile([128, D_MODEL], BF16, tag="wqs")
                # kt by k-chunk, earliest-consumed first, HWDGE queues only
                kq_ = (nc.sync, nc.scalar)
                for k in range(NK):
                    kq_[k % 2].dma_start(
                        kt_sb[:, 2048 * k : 2048 * k + 2048],
                        kt[:, 2048 * k : 2048 * k + 2048],
                    )
                nc.scalar.dma_start(wqg0[:], wq[0])
                nc.sync.dma_start(qt_sb[:, 0:4096], qt[:, 0:4096])
                nc.scalar.dma_start(qt_sb[:, 4096:8192], qt[:, 4096:8192])
                nc.gpsimd.dma_start(bvr_t[:], bvr[:])
                nc.sync.dma_start(wv_sb[:, 0:4160], wv[:, 0:4160])
                nc.scalar.dma_start(wv_sb[:, 4160:8320], wv[:, 4160:8320])

                def emit_kproj(half, g, wkg):
                    # KT[g][:, 1024*half:+1024] = (wk[g].T @ K^T)(half) + bias
                    ps = psS.tile([128, QL], F32, name="ps0", tag="scores")
                    for k in range(NK):
                        for j in range(2):
                            nc.tensor.matmul(
                                ps[:, 512 * j : 512 * j + 512],
                                wkg[:, 128 * k : 128 * k + 128],
                                kt_sb[
                                    :,
                                    2048 * k
                                    + 1024 * half
                                    + 512 * j : 2048 * k
                                    + 1024 * half
                                    + 512 * j
                                    + 512,
                                ],
                                start=(k == 0),
                                stop=(k == NK - 1 and j == 1),
                                skip_group_check=True,
                            )
                    nc.vector.tensor_scalar_add(
                        KT[g][:, 1024 * half : 1024 * half + 1024],
                        ps[:],
                        bkt_t[:, g : g + 1],
                    )

                def emit_qproj(g, wqg):
                    ps = psS.tile([128, QL], F32, name="ps0", tag="scores")
                    for k in range(NK):
                        for j in range(2):
                            nc.tensor.matmul(
                                ps[:, 512 * j : 512 * j + 512],
                                wqg[:, 128 * k : 128 * k + 128],
                                qt_sb[
                                    :, 1024 * k + 512 * j : 1024 * k + 512 * j + 512
                                ],
                                start=(k == 0),
                                stop=(k == NK - 1 and j == 1),
                                skip_group_check=True,
                            )
                    nc.vector.tensor_scalar_add(
                        QT[g][:], ps[:], bqt_t[:, g : g + 1]
                    )

                # pair-0 projections upfront (attention can start ~12us in)
                emit_kproj(0, 0, wkg0)
                emit_kproj(1, 0, wkg0)
                emit_qproj(0, wqg0)

                # ---- fine-grained projection micro-units ------------------
                def v_unit_gen():
                    # one yield = 2 matmuls; a chunk (8 yields) ends with its
                    # bias-add so consumers see a complete v_tiles[c]
                    vts_tiles = {}
                    vts_tiles[0] = vstr.tile([128, 1024], BF16, name="vts0", tag="vts")
                    nc.sync.dma_start(vts_tiles[0][:], vt[0])
                    for c in range(NT):
                        if c + 1 < NT:
                            vts_tiles[c + 1] = vstr.tile(
                                [128, 1024], BF16, name=f"vts{c+1}", tag="vts"
                            )
                            nc.sync.dma_start(vts_tiles[c + 1][:], vt[c + 1])
                        vts = vts_tiles.pop(c)
                        for q in range(4):
                            ps = psP.tile(
                                [128, 260],
                                F32,
                                name="psv",
                                tag="kproj" if q % 2 == 0 else "rep",
                            )
                            for k in range(NK):
                                nc.tensor.matmul(
                                    ps[:],
                                    vts[:, 128 * k : 128 * k + 128],
                                    wv_sb[
                                        :,
                                        VPW * k
                                        + 260 * q : VPW * k
                                        + 260 * q
                                        + 260,
                                    ],
                                    start=(k == 0),
                                    stop=(k == NK - 1),
                                    skip_group_check=True,
                                )
                                if k % 2 == 1 and k < NK - 1:
                                    yield
                            nc.vector.tensor_tensor(
                                v_tiles[c][:, 260 * q : 260 * q + 260],
                                ps[:],
                                bvr_t[:, 260 * q : 260 * q + 260],
                                ADD,
                            )
                            yield
                    v_es.close()

                def pair_unit_gen(g):
                    wkg = wks.tile([128, D_MODEL], BF16, tag="wks")
                    nc.sync.dma_start(wkg[:], wk[g])
                    yield
                    for half in range(2):
                        for j in range(2):
                            ps = psP.tile([128, 512], F32, name="psk", tag="kproj")
                            for k in range(NK):
                                nc.tensor.matmul(
                                    ps[:],
                                    wkg[:, 128 * k : 128 * k + 128],
                                    kt_sb[
                                        :,
                                        2048 * k
                                        + 1024 * half
                                        + 512 * j : 2048 * k
                                        + 1024 * half
                                        + 512 * j
                                        + 512,
                                    ],
                                    start=(k == 0),
                                    stop=(k == NK - 1),
                                    skip_group_check=True,
                                )
                                if k % 2 == 1 and k < NK - 1:
                                    yield
                            nc.vector.tensor_scalar_add(
                                KT[g][
                                    :,
                                    1024 * half + 512 * j : 1024 * half
                                    + 512 * j
                                    + 512,
                                ],
                                ps[:],
                                bkt_t[:, g : g + 1],
                            )
                            yield
                    wqg = wqs.tile([128, D_MODEL], BF16, tag="wqs")
                    nc.scalar.dma_start(wqg[:], wq[g])
                    yield
                    for j in range(2):
                        ps = psP.tile([128, 512], F32, name="psq", tag="kproj")
                        for k in range(NK):
                            nc.tensor.matmul(
                                ps[:],
                                wqg[:, 128 * k : 128 * k + 128],
                                qt_sb[
                                    :, 1024 * k + 512 * j : 1024 * k + 512 * j + 512
                                ],
                                start=(k == 0),
                                stop=(k == NK - 1),
                                skip_group_check=True,
                            )
                            if k % 2 == 1 and k < NK - 1:
                                yield
                        nc.vector.tensor_scalar_add(
                            QT[g][:, 512 * j : 512 * j + 512],
                            ps[:],
                            bqt_t[:, g : g + 1],
                        )
                        yield

                v_gen = v_unit_gen()
                pair_gens = {g: pair_unit_gen(g) for g in range(1, NPAIR)}

                def filler(it):
                    # iteration -> (generator, pumps)
                    if it < 8:
                        # two complete V chunks per iteration (16 yields each)
                        return (v_gen, 32)
                    if it < 16:
                        return (pair_gens[1], 4)
                    g = it // 16 + 1
                    if g < NPAIR:
                        return (pair_gens[g], 2)
                    return (None, 0)

                _STOP = object()

                def pump(gen, n):
                    if gen is None:
                        return
                    for _ in range(n):
                        if next(gen, _STOP) is _STOP:
                            break

                def drain(gen):
                    for _ in gen:
                        pass

                def emit_scores(g, qb, cg):
                    ktg, qtg = KT[g], QT[g]
                    q0 = 512 * qb
                    tiles = [
                        psS.tile([128, QL], F32, name=f"sc{h}", tag="scores")
                        for h in range(2)
                    ]
                    # h0/h1 use disjoint row groups -> can run concurrently;
                    # h-major order gives h1's LDW a 2-MM pull-ahead window
                    for h in range(2):
                        p0 = 64 * h
                        for ci in range(2):
                            c = 2 * cg + ci
                            nc.tensor.matmul(
                                tiles[h][:, 512 * ci : 512 * ci + 512],
                                ktg[p0 : p0 + 64, 128 * c : 128 * c + 128],
                                qtg[p0 : p0 + 64, q0 : q0 + 512],
                                start=True,
                                stop=True,
                                skip_group_check=True,
                            )
                    return tiles

                def emit_spill(g, qb, acc):
                    # X rows -> xg (bf16, SBUF); sum row -> den[g]
                    for h in range(2):
                        if h == 0:
                            # partitions line up: copy straight into xg
                            nc.vector.tensor_copy(
                                xg_tiles[g][0:64, 512 * qb : 512 * qb + 512],
                                acc[h][0:64, :],
                            )
                        else:
                            sx = sgxp.tile([64, 512], BF16, tag="sgx")
                            nc.vector.tensor_copy(sx[:], acc[h][0:64, :])
                            nc.sync.dma_start(
                                xg_tiles[g][64:128, 512 * qb : 512 * qb + 512],
                                sx[:],
                            )
                        sd = sgxp.tile([65, 512], BF16, tag="sgd")
                        nc.vector.tensor_copy(sd[64:65, :], acc[h][64:65, :])
                        nc.sync.dma_start(
                            den[g][2 * h + qb : 2 * h + qb + 1, :], sd[64:65, :]
                        )

                def emit_chain(g):
                    # denr = 1/den ; rep = sel.T @ denr (fp16 broadcast MM)
                    with nc.allow_low_precision(reason="fp16 softmax recips"):
                        nc.vector.reciprocal(denr[g][:], den[g][:])
                    xn = pkq.tile([128, QL], BF16, name=f"xn{g}", tag=f"xn{g}")
                    for qb in range(2):
                        rep = psP.tile([128, 512], F32, name="rep", tag="rep")
                        nc.tensor.matmul(
                            rep[:],
                            sel[qb][:],
                            denr[g][:],
                            start=True,
                            stop=True,
                            skip_group_check=True,
                        )
                        nc.vector.tensor_tensor(
                            xn[:, 512 * qb : 512 * qb + 512],
                            xg_tiles[g][:, 512 * qb : 512 * qb + 512],
                            rep[:],
                            MULT,
                        )
                    xn_tiles[g] = xn

                import os
                if os.environ.get("NOFILL") == "1" or os.environ.get("NOV") == "1":
                    drain(v_gen)
                if os.environ.get("NOFILL") == "1" or os.environ.get("NOPAIR") == "1":
                    for _g in range(1, NPAIR):
                        drain(pair_gens[_g])
                pending_spill = None
                pending_chain = None
                it = 0
                for g in range(NPAIR):
                    if g >= 1:
                        drain(pair_gens[g])  # safety: pair ready before use
                    xg_tiles[g] = pxg.tile(
                        [128, QL], BF16, name=f"xg{g}", tag="xg"
                    )
                    den[g] = denp.tile([4, 512], BF16, name=f"den{g}", tag="den")
                    denr[g] = denp.tile([4, 512], F16, name=f"denr{g}", tag="denr")
                    for qb in range(2):
                        acc = [
                            psacc.tile([65, 512], F32, name="acca", tag="acca"),
                            psacc.tile([65, 512], F32, name="accb", tag="accb"),
                        ]
                        sc_cur = emit_scores(g, qb, 0)
                        if pending_spill is not None:
                            emit_spill(*pending_spill)
                            pending_spill = None
                            if g >= 1 and qb == 1:
                                pending_chain = g - 1
                        for cg in range(NT // 2):
                            gen, n = filler(it)
                            # V chunks must fully precede this iteration's
                            # attnV reads; pair proj units can split around it
                            n1 = n if gen is v_gen else (n + 1) // 2
                            pump(gen, n1)
                            sc_next = (
                                emit_scores(g, qb, cg + 1)
                                if cg + 1 < NT // 2
                                else None
                            )
                            exs = []
                            for h in range(2):
                                ex = expp.tile([128, QL], BF16, tag="exp")
                                nc.scalar.activation(
                                    ex[:], sc_cur[h][:], AF.Exp, scale=0.125
                                )
                                exs.append(ex)
                            for h in range(2):
                                hh = 2 * g + h
                                for ci in range(2):
                                    c = 2 * cg + ci
                                    nc.tensor.matmul(
                                        acc[h][:],
                                        v_tiles[c][:, 65 * hh : 65 * hh + 65],
                                        exs[h][:, 512 * ci : 512 * ci + 512],
                                        start=(c == 0),
                                        stop=(c == NT - 1),
                                        skip_group_check=True,
                                    )
                                if h == 0:
                                    pump(gen, n - n1)
                            if pending_chain is not None and cg == 2:
                                emit_chain(pending_chain)
                                pending_chain = None
                            it += 1
                            sc_cur = sc_next
                        pending_spill = (g, qb, acc)
                emit_spill(*pending_spill)
                for g in range(1, NPAIR):
                    drain(pair_gens[g])
                drain(v_gen)
                kq_es.close()
                emit_chain(7)

            # ---- output projection ---------------------------------------
            with (
                P(name="pwo", bufs=1) as pwo,
                P(name="ps3o", bufs=6, space="PSUM") as ps3o,
            ):
                bor_t = consts.tile([128, D_MODEL], BF16, tag="bor")
                nc.scalar.dma_start(bor_t[:], bor[:])
                wo_sb = pwo.tile([128, NPAIR * D_MODEL], BF16, tag="wosb")
                for g in range(NPAIR):
                    (nc.sync, nc.scalar)[g % 2].dma_start(
                        wo_sb[:, 1024 * g : 1024 * g + 1024],
                        wo[:, 1024 * g : 1024 * g + 1024],
                    )

                qrr = 0
                for m in range(QL // 128):
                    for j in range(2):
                        ps = ps3o.tile([128, 512], F32, tag="oproj")
                        for g in range(NPAIR):
                            nc.tensor.matmul(
                                ps[:],
                                xn_tiles[g][:, 128 * m : 128 * m + 128],
                                wo_sb[
                                    :,
                                    1024 * g + 512 * j : 1024 * g + 512 * j + 512,
                                ],
                                start=(g == 0),
                                stop=(g == NPAIR - 1),
                                skip_group_check=True,
                            )
                        ot = stg.tile([128, 512], F32, tag="outs")
                        nc.vector.tensor_tensor(
                            ot[:], ps[:], bor_t[:, 512 * j : 512 * j + 512], ADD
                        )
                        eng = (nc.sync, nc.scalar)[qrr % 2]
                        qrr += 1
                        eng.dma_start(
                            out[128 * m : 128 * m + 128, 512 * j : 512 * j + 512],
                            ot[:],
                        )


_NC_CACHE = None
LAST_RESULT = None


def _get_nc():
    global _NC_CACHE
    if _NC_CACHE is None:
        _install_patch()
        _NC_CACHE = _build_bass()
    return _NC_CACHE


def kernel(q, k, v, w_q, b_q, w_k, b_k, w_v, b_v, w_o, b_o):
    global LAST_RESULT
    import ml_dtypes

    q = np.asarray(q, np.float32)
    k = np.asarray(k, np.float32)
    v = np.asarray(v, np.float32)

    def _pair_w(w):
        # [in, out] -> [g, 128, 1024]: [g][p, 128k+j] = w[128k+p, 128g+j]
        return np.ascontiguousarray(
            np.asarray(w, np.float32)
            .reshape(NK, 128, NPAIR, 128)
            .transpose(2, 1, 0, 3)
            .reshape(NPAIR, 128, D_MODEL)
        ).astype(ml_dtypes.bfloat16)

    def _chunk_w(w):
        # [in, out] -> [128, 8*1024]: [p, 1024k+o] = w[128k+p, o]
        return np.ascontiguousarray(
            np.asarray(w, np.float32)
            .reshape(NK, 128, D_MODEL)
            .transpose(1, 0, 2)
            .reshape(128, NK * D_MODEL)
        ).astype(ml_dtypes.bfloat16)

    w_q = _pair_w(w_q)
    w_k = _pair_w(w_k)
    # wv padded per chunk to 16 heads x 65 cols; 65th col w=0 (ones come
    # from the padded bias), so the on-chip proj writes are contiguous
    wv_c = np.asarray(w_v, np.float32).reshape(NK, 128, 16, 64)
    wv_pad = np.zeros((128, NK, 16, 65), np.float32)
    wv_pad[:, :, :, 0:64] = wv_c.transpose(1, 0, 2, 3)
    w_v = np.ascontiguousarray(wv_pad.reshape(128, NK * VPW)).astype(
        ml_dtypes.bfloat16
    )
    # wo: [p, 1024g+o] = w_o[128g+p, o] -- same transform (g indexes chunks)
    w_o = _chunk_w(w_o)
    b_q = np.asarray(b_q, np.float32)
    b_k = np.asarray(b_k, np.float32)
    b_v = np.asarray(b_v, np.float32)
    b_o = np.asarray(b_o, np.float32)

    bqt = np.ascontiguousarray(b_q.reshape(NK, 128).T)
    bkt = np.ascontiguousarray(b_k.reshape(NK, 128).T)
    bvr_pad = np.ones((16, 65), np.float32)
    bvr_pad[:, 0:64] = b_v.reshape(16, 64)
    bvr = np.ascontiguousarray(
        np.broadcast_to(bvr_pad.reshape(1, VPW), (128, VPW))
    ).astype(ml_dtypes.bfloat16)
    bor = np.ascontiguousarray(
        np.broadcast_to(b_o[None, :], (128, D_MODEL))
    ).astype(ml_dtypes.bfloat16)
    selq = np.zeros((2, 4, 128), np.float16)
    for qb in range(2):
        selq[qb, qb, 0:64] = 1.0
        selq[qb, 2 + qb, 64:128] = 1.0

    in_maps = []
    for c in range(N_CORES):
        b = c // 2
        r0 = QL * (c % 2)
        # qt: [p, 1024k+t] = q_proj_input^T chunked
        qtc = np.ascontiguousarray(
            q[b, r0 : r0 + QL, :].T.reshape(NK, 128, QL).transpose(1, 0, 2).reshape(
                128, NK * QL
            )
        ).astype(ml_dtypes.bfloat16)
        ktc = np.ascontiguousarray(
            k[b].T.reshape(NK, 128, S).transpose(1, 0, 2).reshape(128, NK * S)
        ).astype(ml_dtypes.bfloat16)
        in_maps.append(
            {
                "qt": qtc,
                "kt": ktc,
                "vt": np.ascontiguousarray(
                    v[b]
                    .T.reshape(8, 128, 16, 128)
                    .transpose(2, 1, 0, 3)
                    .reshape(16, 128, 1024)
                ).astype(ml_dtypes.bfloat16),
                "wq": w_q,
                "wk": w_k,
                "wv": w_v,
                "wo": w_o,
                "bqt": bqt,
                "bkt": bkt,
                "bvr": bvr,
                "bor": bor,
                "selq": selq,
            }
        )

    nc = _get_nc()
    res = run_bass_kernel_spmd(nc, in_maps, list(range(N_CORES)))
    LAST_RESULT = res

    outp = np.empty((B, S, D_MODEL), np.float32)
    for c in range(N_CORES):
        b = c // 2
        r0 = QL * (c % 2)
        outp[b, r0 : r0 + QL, :] = res.results[c]["out"]
    return outp
